# revision 5
# baseline (speedup 1.0000x reference)
"""Trainium2 Bass kernel: modulated conv2d via 1-D Winograd F(2,3) along W.

Reference math (StyleGAN-style modulated conv, style[0] only):
    w  = kernel * he_std; s = style @ w_mod + b_mod + 1; s /= max|s|
    w  = w * s[0][...]; w *= rsqrt(sum(w^2,(0,1,2)) + 1e-8)
    y  = lrelu_0.2(conv2d_same(x, w) + noise*(ns/2) + bias) * sqrt(2)

Only style[0] modulates, so all 8 batch images share one effective weight.
The weight math, bias/noise add, lrelu and sqrt2 are pointwise/host-cheap;
the device computes the pure conv via Winograd, data-parallel over batch.

Device algorithm (per core, 1 image, NCHW):
  * 1-D Winograd F(2,3) along W cuts PE work 1.5x: per output col pair t,
    d_j = x_pad[., 2t+j], comps V = B^T d, M_k = sum_dh G_k,dh (x) V_k
    (matmul over cin), z_even = M0+M1+M2, z_odd = M1-M2-M3.
  * Host pre-pads x (SAME) and de-interleaves cols [Ep(129)|Op(129)] per
    row, so the V transform is 3 unit-stride bf16 tensor_tensor ops per
    slab (DVE 2x mode); v0/v3 share one subtract over the whole row.
  * Per 4-row group: 12 matmuls (4 comps x 3 vertical taps), rhs = V rows
    [q+dh : q+dh+4] comp k (2D AP), PSUM [cout, 4 banks].
  * Epilogue: ACT extracts all 4 banks in ONE wide op (PSUM fp32 -> SBUF
    bf16), then 4 DVE bf16 tensor_tensor ops (2x mode): t1 = m1+m2,
    t2 = m1-m2, z_e = t1+m0, z_o = t2-m3 (written straight into the
    [row, even|odd, 128] output tile).
  * Output z [cout, H, 2, 128] bf16; host applies bias+noise+lrelu*sqrt2
    and re-interleaves to NHWC fp32.
"""

from contextlib import ExitStack

import ml_dtypes
import numpy as np

import concourse.bacc as bacc
import concourse.mybir as mybir
import concourse.tile as tile
from concourse.alu_op_type import AluOpType
from concourse.bass_utils import run_bass_kernel_spmd

B, H, W, CIN, COUT, KK, SDIM = 8, 256, 256, 128, 128, 3, 512
HP = H + 2  # padded rows
WP = W + 2  # de-interleaved row length: Ep(129) | Op(129)
W2 = W // 2
VROW = 517  # V row: [v0(128) | j(1) | v3(128) | v1(128) | v2(128)]
V_OFF = (0, 257, 385, 129)  # comp k -> col offset within V row
N_CORES = 8
# ramp/tail-optimized slab schedule: small first slabs so the PE starts
# almost immediately; small last slab to drain the pipeline faster
SLAB_ROWS = (4, 8, 16, 24, 32, 32, 32, 32, 32, 32, 8, 2, 2)
assert sum(SLAB_ROWS) == H
MAX_V_ROWS = 34
GROUP_ROWS = 4

BF16 = mybir.dt.bfloat16
F32 = mybir.dt.float32
SQRT2 = float(np.sqrt(np.float32(2.0)))
ADD = AluOpType.add
SUB = AluOpType.subtract

USE_GP = False  # offload one epilogue op per group to GpSimd


def _effective_weight(style, kernel, w_mod, b_mod):
    """Exactly the reference weight math, in fp32 numpy."""
    style = np.asarray(style, np.float32)
    kernel = np.asarray(kernel, np.float32)
    w_mod = np.asarray(w_mod, np.float32)
    b_mod = np.asarray(b_mod, np.float32)

    he_std = np.float32(1.0) / np.sqrt(np.float32(KK * KK * CIN))
    w = kernel * he_std
    s = (style @ w_mod + b_mod + np.float32(1.0)).astype(np.float32)
    s = s * (np.float32(1.0) / np.max(np.abs(s)))
    w = w * s[0][None, None, :, None]
    d = np.float32(1.0) / np.sqrt(
        np.sum(np.square(w), axis=(0, 1, 2), dtype=np.float32) + np.float32(1e-8)
    )
    w = w * d[None, None, None, :]
    return w.astype(np.float32)  # [3, 3, cin, cout]


def _build_program():
    nc = bacc.Bacc(trn_type="TRN2")
    x = nc.declare_dram_parameter("x", [CIN, HP * WP], BF16, isOutput=False)
    g = nc.declare_dram_parameter("g", [CIN, 12 * COUT], BF16, isOutput=False)
    y = nc.declare_dram_parameter("y", [COUT, H * 2 * W2], BF16, isOutput=True)

    with ExitStack() as ctx:
        tc = ctx.enter_context(tile.TileContext(nc))
        consts = ctx.enter_context(tc.tile_pool(name="consts", bufs=1))
        xpool = ctx.enter_context(tc.tile_pool(name="x", bufs=3))
        vpool = ctx.enter_context(tc.tile_pool(name="v", bufs=2))
        pspool = ctx.enter_context(tc.tile_pool(name="ps", bufs=2, space="PSUM"))
        cpool = ctx.enter_context(tc.tile_pool(name="c", bufs=3))
        tpool = ctx.enter_context(tc.tile_pool(name="t", bufs=3))
        opool = ctx.enter_context(tc.tile_pool(name="o", bufs=3))

        # first two x chunks queued before the weights: they gate the first
        # matmuls of slabs 0/1 while the PE is still warming up
        n0 = SLAB_ROWS[0] + 2
        xt0 = xpool.tile([CIN, MAX_V_ROWS * WP], BF16, tag="x", bufs=3)
        nc.sync.dma_start(xt0[:, : n0 * WP], x[:, : n0 * WP])
        n1 = SLAB_ROWS[1] + 2
        r1 = SLAB_ROWS[0]
        xt1 = xpool.tile([CIN, MAX_V_ROWS * WP], BF16, tag="x", bufs=3)
        nc.sync.dma_start(xt1[:, : n1 * WP], x[:, r1 * WP : (r1 + n1) * WP])
        gt = consts.tile([CIN, 12 * COUT], BF16)
        nc.sync.dma_start(gt[:], g[:])

        # dummy matmuls ramp the PE clock out of its low p-state while the
        # first x chunk is still in flight; a memset-sourced tile avoids
        # waiting on any DMA
        dummy = consts.tile([CIN, 512], BF16)
        nc.vector.memset(dummy[:], 0)
        warm = pspool.tile([COUT, 4 * 512], F32, tag="ps")
        for _ in range(22):
            nc.tensor.matmul(warm[:, 0:512], dummy[:, 0:COUT], dummy[:],
                             start=True, stop=True)

        r0 = 0
        for slab_i, rows in enumerate(SLAB_ROWS):
            n_vrows = rows + 2
            grows = min(GROUP_ROWS, rows)  # 2-row groups for tiny tail slabs
            fs = grows * 128
            groups = rows // grows
            if slab_i == 0:
                xt = xt0
            elif slab_i == 1:
                xt = xt1
            else:
                xt = xpool.tile([CIN, MAX_V_ROWS * WP], BF16, tag="x", bufs=3)
                nc.sync.dma_start(
                    xt[:, : n_vrows * WP], x[:, r0 * WP : (r0 + n_vrows) * WP])
            xv = xt[:].rearrange("p (r c) -> p r c", c=WP)

            vt = vpool.tile([CIN, MAX_V_ROWS * VROW], BF16, tag="v", bufs=3)
            vv = vt[:].rearrange("p (r c) -> p r c", c=VROW)
            nr = n_vrows

            def transform_rows(a, b):
                # Ep[t]=xv[t] (t<129), Op[t]=xv[129+t]
                # v0 = Ep[t]-Ep[t+1] and v3 = Op[t]-Op[t+1]: one subtract
                nc.vector.tensor_sub(
                    vv[:, a:b, 0:257], xv[:, a:b, 0:257], xv[:, a:b, 1:258])
                # v1 = Op[t] + Ep[t+1]
                nc.vector.tensor_add(
                    vv[:, a:b, 257:385], xv[:, a:b, 129:257],
                    xv[:, a:b, 1:129])
                # v2 = Ep[t+1] - Op[t]
                nc.vector.tensor_sub(
                    vv[:, a:b, 385:513], xv[:, a:b, 1:129],
                    xv[:, a:b, 129:257])

            # transform in chunks interleaved with the group loop so the
            # DVE never blocks group epilogues for long; chunk j (rows up
            # to 8j+9) lands just before group 2j needs it
            chunk_ends = [min(e, nr) for e in (10, 18, 26, 34)]
            transform_rows(0, chunk_ends[0])
            done = chunk_ends[0]

            ot = None
            for gi in range(groups):
                if gi % 2 == 0 and gi // 2 + 1 < len(chunk_ends):
                    e = chunk_ends[gi // 2 + 1]
                    if e > done:
                        transform_rows(done, e)
                        done = e
                q = gi * grows
                ps = pspool.tile([COUT, 4 * 512], F32, tag="ps")
                for k in range(4):
                    for dh in range(3):
                        nc.tensor.matmul(
                            ps[:, k * 512 : k * 512 + fs],
                            gt[:, (k * 3 + dh) * COUT : (k * 3 + dh + 1) * COUT],
                            vv[:, q + dh : q + dh + grows,
                               V_OFF[k] : V_OFF[k] + 128],
                            start=(dh == 0),
                            stop=(dh == 2),
                        )

                # extraction: fp32 PSUM [m0|m1|m2|m3] -> bf16 SBUF. One wide
                # op in steady state; per-bank pieces for the tiny tail
                # slabs so the DVE chain starts earlier (shorter drain).
                ct = cpool.tile([COUT, 4 * 512], BF16)
                ident = mybir.ActivationFunctionType.Identity
                if rows == 2:
                    for k in (1, 2, 0, 3):
                        nc.scalar.activation(
                            ct[:, k * 512 : k * 512 + fs],
                            ps[:, k * 512 : k * 512 + fs],
                            ident, bias=0.0, scale=1.0)
                else:
                    nc.scalar.activation(ct[:], ps[:], ident,
                                         bias=0.0, scale=1.0)
                c0 = ct[:, 0:fs]
                c1 = ct[:, 512 : 512 + fs]
                c2 = ct[:, 1024 : 1024 + fs]
                c3 = ct[:, 1536 : 1536 + fs]

                t1 = tpool.tile([COUT, 512], BF16, tag="t1")
                nc.vector.tensor_add(t1[:, :fs], c1, c2)
                t2 = tpool.tile([COUT, 512], BF16, tag="t2")
                nc.vector.tensor_sub(t2[:, :fs], c1, c2)

                if ot is None:
                    ot = opool.tile([COUT, 8 * 256], BF16)
                    ov = ot[:].rearrange("p (r h c) -> p r h c", h=2, c=128)
                    base = gi
                half = (gi - base) * grows
                nc.vector.tensor_add(
                    ov[:, half : half + grows, 0, :], t1[:, :fs], c0)
                eng = nc.gpsimd if USE_GP else nc.vector
                eng.tensor_sub(
                    ov[:, half : half + grows, 1, :], t2[:, :fs], c3)
                if gi - base == 1 or gi == groups - 1:
                    row = r0 + base * grows
                    n_out = (gi - base + 1) * grows
                    nc.sync.dma_start(
                        y[:, row * 256 : (row + n_out) * 256],
                        ot[:, : n_out * 256])
                    ot = None
            r0 += rows
    nc.finalize()
    return nc


def _prep_inputs(inputs):
    x = np.asarray(inputs["x"])
    w_eff = _effective_weight(
        inputs["style"], inputs["kernel"], inputs["w_mod"], inputs["b_mod"]
    )
    # Winograd weight transform along W (kw):
    #   G0=w0, G1=(w0+w1+w2)/2, G2=(w0-w1+w2)/2, G3=w2   per dh
    w0, w1, w2 = w_eff[:, 0], w_eff[:, 1], w_eff[:, 2]  # [dh, cin, cout]
    gs = np.stack(
        [w0, (w0 + w1 + w2) * 0.5, (w0 - w1 + w2) * 0.5, w2], axis=0
    ).astype(np.float32)  # [k, dh, cin, cout]
    g_dev = np.ascontiguousarray(
        gs.reshape(12, CIN, COUT).transpose(1, 0, 2).reshape(CIN, 12 * COUT)
    ).astype(ml_dtypes.bfloat16)

    # x: NHWC -> per-image [cin, 258 rows, 258] bf16, padded + de-interleaved
    xc = x.transpose(0, 3, 1, 2).astype(np.float32)  # [B, cin, H, W]
    x_pad = np.zeros((B, CIN, HP, W + 2), dtype=np.float32)
    x_pad[:, :, 1 : H + 1, 1 : W + 1] = xc
    x_d = np.empty((B, CIN, HP, WP), dtype=ml_dtypes.bfloat16)
    x_d[:, :, :, 0:129] = x_pad[:, :, :, 0:258:2]
    x_d[:, :, :, 129:258] = x_pad[:, :, :, 1:258:2]

    in_maps = [
        {"x": np.ascontiguousarray(x_d[b].reshape(CIN, HP * WP)), "g": g_dev}
        for b in range(B)
    ]
    return in_maps


def _run(inputs, trace=False, **spmd_kwargs):
    in_maps = _prep_inputs(inputs)
    nc = _build_program()
    res = run_bass_kernel_spmd(
        nc, in_maps, list(range(N_CORES)), trace=trace, **spmd_kwargs
    )

    noise_strength = float(np.asarray(inputs["noise_strength"]).reshape(-1)[0])
    bias = np.asarray(inputs["bias"], np.float32)
    noise = np.asarray(inputs["noise"], np.float32)  # [B, H, W, 1]

    out = np.empty((B, H, W, COUT), dtype=np.float32)
    for b in range(B):
        zb = res.results[b]["y"].reshape(COUT, H, 2, W2).astype(np.float32)
        z = zb.transpose(1, 3, 2, 0).reshape(H, W, COUT)  # [H, W, COUT]
        z = z + noise[b] * np.float32(noise_strength / 2.0)
        z = z + bias
        out[b] = np.where(z >= 0, z, np.float32(0.2) * z) * np.float32(SQRT2)
    return out, res


def kernel(**inputs):
    out, _ = _run(inputs)
    return out
